# revision 2
# baseline (speedup 1.0000x reference)
"""Distributed multi-head attention kernel for 8 TRN2 NeuronCores (v2).

Problem: x[2,2048,1024] -> qkv proj (w_qkv[3072,1024]) -> 16-head SDPA ->
out proj (w_proj[1024,1024], b_proj[1024]).

Sharding: tensor-parallel over heads. Core c owns heads {2c, 2c+1}:
  - stage 1 (per core): q/k/v for its 2 heads over ALL 4096 tokens,
    transposed score tiles S^T[m,n] per (batch, head), exp on the scalar
    engine (no max-subtraction: scores ~ N(0,1), fp32 exp is safe), PV with
    a trailing ones-column in V so PSUM row DH accumulates the softmax
    denominator, then normalize. Result: aT_h [64 head-dims, 4096 tokens].
  - Four AllToAlls (one per half-batch, all but the last overlapped with
    compute) reshard from head-parallel to token-parallel.
  - stage 2 (per core): y^T[1024, 512] = w_proj @ a + b_proj for its shard.

v2 changes vs v1 (all bf16 on the TensorE; fp8 was tried and rejected —
softmax weight noise passes ~1:1 to the output, it does NOT average out):
  * The Activation ring issues NO DMAs: exp is the only Act-engine work.
    Previously a collective-gated agT read on the Act ring head-of-line
    blocked the exps of the next batch, stalling the whole pipeline for
    the full AllToAll duration (~20 us, twice).
  * All DMAs ride the SP ring (x, weights partition-major with the 2MB
    wp load deferred past all x-chunk issues, consolidated a2a-in
    writes, split agT gathers, consolidated y writes); the Pool ring
    carries ONLY the AllToAlls.
  * gpsimd partition_broadcast is replaced by a 1-contraction PE matmul
    (ones stationary at tile_position (64,0)) so the normalize never
    queues behind an in-flight collective on the Pool ring; it is
    deferred into m-tile 2 of the next chunk so nothing waits on the
    DVE reciprocal.
  * The ENTIRE output projection runs at the tail, inside the final
    AllToAll's latency window (the PE is otherwise idle there and every
    agT gather except the final one completed chunks earlier).
    Interleaving proj bursts into batch-1's attention instead stretches
    its Act-paced exp pipeline chunk for chunk, and proj pieces that
    outrun their agT data wedge the PE's 4-deep wait queue.

All TensorE matmuls run in bf16 (fp32 PSUM accumulation); softmax exp is
computed in fp32 on the scalar engine straight out of PSUM; 1/denominator
is bf16 (0.2% scale noise, inside the error budget).
"""

import numpy as np
import ml_dtypes

import concourse.bass as bass
import concourse.bacc as bacc
import concourse.tile as tile
import concourse.mybir as mybir
from concourse import bass_utils

BF16 = mybir.dt.bfloat16
F32 = mybir.dt.float32
FP8 = mybir.dt.float8e4
DR = mybir.MatmulPerfMode.DoubleRow
NP_FP8 = ml_dtypes.float8_e4m3

N_CORES = 8
B = 2
N = 2048
DIM = 1024
H = 16
DH = 64
SCALE = DH ** -0.5
HPC = H // N_CORES          # heads per core = 2
T = B * N                   # 4096 global tokens
TPC = T // N_CORES          # 512 tokens per core in stage 2
SPB = TPC // B              # stage-2 tokens per core per batch = 256
SPP = SPB // 2              # tokens per shard piece = 128
CT = DIM // 128             # 8 contraction tiles
CP = CT // 2                # 4 contraction pair-tiles for DoubleRow
TCH = 512                   # token chunk for stage-1 matmul streaming
NCH = 512                   # n (query) chunk in attention
MT = N // 128               # 16 m-tiles per batch

_cached = None


class _Ctx:
    pass


def _load_chunk(c, tci):
    """DMA one token chunk of x^T into SBUF, split per c-tile pair so the
    first matmul can start after 1/4 of the transfer."""
    t0 = tci * TCH
    xc = c.xin.tile([128, CT, TCH], BF16, tag="xc", name="xc")
    for a in range(CP):
        c.nc.sync.dma_start(
            xc[:, 2 * a:2 * a + 2, :],
            c.xT_d[256 * a:256 * (a + 1), t0:t0 + TCH]
            .rearrange("(t p) n -> p t n", p=128))
    return xc


def _k_chunk(c, xc, tci):
    t0 = tci * TCH
    k_ps = c.psA.tile([128, TCH], F32, tag="psA", name="k_ps")
    for a in range(CT):
        c.nc.tensor.matmul(k_ps[:], c.wk_sb[:, a, :], xc[:, a, :],
                           start=(a == 0), stop=(a == CT - 1))
    c.nc.vector.tensor_copy(c.kT[:, t0:t0 + TCH], k_ps[:])


def _q_chunk(c, xc, tci):
    t0 = tci * TCH
    q_ps = c.psA.tile([128, TCH], F32, tag="psA", name="q_ps")
    for a in range(CT):
        c.nc.tensor.matmul(q_ps[:], c.wq_sb[:, a, :], xc[:, a, :],
                           start=(a == 0), stop=(a == CT - 1))
    c.nc.vector.tensor_copy(c.qT[:, t0:t0 + TCH], q_ps[:])


def _v_chunk(c, xc, tci):
    t0 = tci * TCH
    for mt in range(TCH // 128):
        gmt = (t0 // 128) + mt
        v_ps = c.psA.tile([128, 128], F32, tag="psA", name="v_ps")
        for a in range(CT):
            c.nc.tensor.matmul(
                v_ps[:], xc[:, a, 128 * mt:128 * (mt + 1)], c.wv_sb[:, a, :],
                start=(a == 0), stop=(a == CT - 1))
        for h in range(HPC):
            c.nc.vector.tensor_copy(
                c.v_aug[h][:, gmt, 0:DH], v_ps[:, DH * h:DH * (h + 1)])


def _qv_chunk(c, xc, tci):
    _q_chunk(c, xc, tci)
    _v_chunk(c, xc, tci)


def _attn_qk_exp(c, b, nci, mt):
    nc = c.nc
    n0 = b * N + nci * NCH
    m0 = b * N + 128 * mt
    s_ps = c.psA.tile([128, HPC * NCH], F32, tag="psA", name="s_ps")
    e_t = c.etp.tile([128, HPC * NCH], BF16, tag="et", name="e_t")
    for h in range(HPC):
        nc.tensor.matmul(
            s_ps[:, NCH * h:NCH * (h + 1)],
            c.kT[DH * h:DH * (h + 1), m0:m0 + 128],
            c.qT[DH * h:DH * (h + 1), n0:n0 + NCH],
            start=True, stop=True,
            tile_position=(DH * h, 0))
    nc.scalar.activation(e_t[:], s_ps[:],
                         mybir.ActivationFunctionType.Exp, scale=SCALE)
    return e_t


def _attn_pv(c, mt, gmt, e_t, o_ps):
    nc = c.nc
    for h in range(HPC):
        nc.tensor.matmul(
            o_ps[h][:], c.v_aug[h][:, gmt, :],
            e_t[:, NCH * h:NCH * (h + 1)],
            start=(mt == 0), stop=(mt == MT - 1))


def _norm_recip(c, b, nci, o_ps):
    """Row DH of o_ps is the softmax denominator (ones-column of v_aug);
    the reciprocal stays at partition 64 — engine lanes are physical, so
    in/out partition ranges must match and start 0/32/64/96-aligned.
    bf16 out: a 0.2% scale noise per token, inside the error budget, and
    it keeps the broadcast matmul at full bf16 speed."""
    nc = c.nc
    den = c.small.tile([DH + 1, HPC * NCH], BF16, tag="den", name="den")
    with nc.allow_low_precision(reason="bf16 1/den: 0.2% scale noise ok"):
        for h in range(HPC):
            nc.vector.reciprocal(den[DH:DH + 1, NCH * h:NCH * (h + 1)],
                                 o_ps[h][DH:DH + 1, :])
    return den


def _norm_apply(c, pend):
    """Broadcast 1/den from partition 64 across partitions 0..DH-1 with a
    1-contraction PE matmul at tile_position (64, 0) (no gpsimd: the Pool
    ring is collectives-only so AllToAlls never head-of-line block the
    broadcast; no DMA: the Act ring stays exp-only), then scale rows
    0..DH-1 of o_ps into aT. Deferred into m-tile 2 of the next chunk so
    the PE matmul never waits on the DVE reciprocal."""
    nc = c.nc
    b, nci, o_ps, den = pend
    n0 = b * N + nci * NCH
    rb = c.small.tile([DH, HPC * NCH], F32, tag="rb", name="rb")
    rb_ps = c.psA.tile([DH, HPC, NCH], F32, tag="psA", name="rb_ps")
    for h in range(HPC):
        nc.tensor.matmul(rb_ps[:, h, :], c.ones_sb[DH:DH + 1, :],
                         den[DH:DH + 1, NCH * h:NCH * (h + 1)],
                         start=True, stop=True, tile_position=(DH, 0))
    # DVE reads at most one PSUM operand per op: stage rb in SBUF,
    # per head so each mul (and the a2a write behind it) starts as soon
    # as its half of the staging copy lands
    for h in range(HPC):
        nc.vector.tensor_copy(rb[:, NCH * h:NCH * (h + 1)], rb_ps[:, h, :])
        nc.vector.tensor_mul(c.aT[h][:, n0:n0 + NCH],
                             o_ps[h][0:DH, :],
                             rb[:, NCH * h:NCH * (h + 1)])
    # aT columns for this chunk are final: stage its reshard pieces
    half, sub = divmod(nci, 2)
    if sub == 1:
        _a2a_in_writes(c, b, half)
        _reshard_half(c, b, half)


def _prefetch_qk(c, b, nci, mt):
    """Compute one of a later n-chunk's QK^T+exp tiles now, in this
    chunk's Activation-idle window, and stash it for the PV loop."""
    def f():
        c.prefetch[(b, nci, mt)] = _attn_qk_exp(c, b, nci, mt)
    return f


def _attn_nchunk(c, b, nci, interleave=None):
    """One attention n-chunk; optional {mt: fn} callbacks emitted after
    given m-tiles to fill TensorEngine slack. The previous chunk's
    normalize (PE broadcast + DVE scale) is emitted after m-tile 1."""
    o_ps = [c.psB.tile([DH + 1, NCH], F32, tag="psB", name=f"o_ps{h}")
            for h in range(HPC)]
    # software-pipelined two deep: PV of m-tile mt-2 is emitted after
    # QK^T/exp of m-tile mt, so the next QK^T never queues behind an
    # exp-blocked PV, and the first PV (which waits for the previous
    # chunk's o_ps slot, released by its normalize-muls) has two QK^T
    # pairs in front of it to cover the normalize latency.
    pv_q = []
    for mt in range(MT):
        e_t = c.prefetch.pop((b, nci, mt), None)
        if e_t is None:
            e_t = _attn_qk_exp(c, b, nci, mt)
        if mt == 2 and c.pending_norm is not None:
            _norm_apply(c, c.pending_norm)
            c.pending_norm = None
        if len(pv_q) >= 4:
            pmt, pe_t = pv_q.pop(0)
            _attn_pv(c, pmt, b * MT + pmt, pe_t, o_ps)
        pv_q.append((mt, e_t))
        if interleave and mt in interleave:
            interleave[mt]()
    for pmt, pe_t in pv_q:
        _attn_pv(c, pmt, b * MT + pmt, pe_t, o_ps)
    den = _norm_recip(c, b, nci, o_ps)
    c.pending_norm = (b, nci, o_ps, den)


def _a2a_in_writes(c, b, half):
    """Write this half-batch's aT columns to the a2a staging buffer: one
    DMA per head on the DVE ring (dest iterates (d, j, t) to match the
    natural SBUF source order)."""
    t0 = b * N + half * (N // 2)
    for h in range(HPC):
        c.nc.sync.dma_start(
            c.a2a_in[b][half][:, h, :, :].rearrange("j d t -> d j t"),
            c.aT[h][:, t0:t0 + N // 2])


def _reshard_half(c, b, half):
    """AllToAll for half-batch (b, half): core j gets C-complete rows for
    its SPP-token piece [b*N + half*N/2 + SPP*j, +SPP). The agT gather
    rides the SP ring so it never blocks exps behind the collective."""
    nc = c.nc
    if c.use_collective:
        nc.gpsimd.collective_compute(
            "AllToAll", mybir.AluOpType.bypass,
            replica_groups=[list(range(N_CORES))],
            ins=[c.a2a_in[b][half].opt()], outs=[c.a2a_out[b][half].opt()])
    else:
        nc.sync.dma_start(c.a2a_out[b][half][:], c.a2a_in[b][half][:])
    col0 = b * SPB + half * SPP
    # split the gather so the projection's first a-tiles can start after
    # a fraction of the transfer; the final (fully latency-exposed)
    # gather is split 4 ways
    ns = 4 if (b, half) == (1, 1) else 2
    step = CT // ns
    for i in range(ns):
        nc.sync.dma_start(
            c.agT[:, i * step:(i + 1) * step, col0:col0 + SPP],
            c.a2a_out[b][half][i * step:(i + 1) * step]
            .rearrange("w h d t -> (h d) w t"))


def _proj_cols(c, col0, ncols, ots):
    """Output projection for agT columns [col0, col0+ncols) into y_sb."""
    nc = c.nc
    for ot in ots:
        y_ps = c.psA.tile([128, SPB], F32, tag="psA", name="y_ps")
        for a in range(CT):
            nc.tensor.matmul(
                y_ps[:, 0:ncols], c.wp_sb[:, a, 128 * ot:128 * (ot + 1)],
                c.agT[:, a, col0:col0 + ncols],
                start=(a == 0), stop=(a == CT - 1))
        nc.vector.tensor_scalar_add(c.y_sb[:, ot, col0:col0 + ncols],
                                    y_ps[:, 0:ncols],
                                    c.bmat_sb[:, ot:ot + 1])


def _out_write(c, col0, ncols, ots=(0, CT)):
    o0, o1 = ots
    c.nc.sync.dma_start(
        c.out_d[128 * o0:128 * o1, col0:col0 + ncols]
        .rearrange("(o p) t -> p o t", p=128),
        c.y_sb[:, o0:o1, col0:col0 + ncols])


def _build(use_collective=True, reps=1, num_devices=None):
    """reps>1 unrolls the whole computation N times inside one NEFF —
    used only for timing (differencing out per-execution overhead)."""
    if num_devices is None:
        num_devices = N_CORES if use_collective else 1
    nc = bacc.Bacc("TRN2", target_bir_lowering=False, debug=False,
                   num_devices=num_devices)
    c = _Ctx()
    c.nc = nc
    c.use_collective = use_collective

    c.xT_d = nc.dram_tensor("xT", [DIM, T], BF16, kind="ExternalInput")
    # weights arrive partition-major ([128, CT*...]) so their DMAs are 128
    # full-row descriptors instead of 1024 sub-512B ones
    wqT_d = nc.dram_tensor("wqT", [128, CT * 128], BF16, kind="ExternalInput")
    wkT_d = nc.dram_tensor("wkT", [128, CT * 128], BF16, kind="ExternalInput")
    wvT_d = nc.dram_tensor("wvT", [128, CT * 128], BF16, kind="ExternalInput")
    wpT_d = nc.dram_tensor("wpT", [128, CT * DIM], BF16, kind="ExternalInput")
    bmat_d = nc.dram_tensor("bmat", [128, CT], F32, kind="ExternalInput")
    c.out_d = nc.dram_tensor("out", [DIM, TPC], BF16, kind="ExternalOutput")

    with tile.TileContext(nc) as tc:
        with (
            tc.tile_pool(name="const", bufs=1) as const,
            tc.tile_pool(name="xin", bufs=7) as xin,
            tc.tile_pool(name="acts", bufs=1) as acts,
            tc.tile_pool(name="et", bufs=16) as etp,
            tc.tile_pool(name="small", bufs=3) as small,
            tc.tile_pool(name="psA", bufs=3, space="PSUM") as psA,
            tc.tile_pool(name="psB", bufs=2, space="PSUM") as psB,
            tc.tile_pool(name="dram", bufs=1, space="DRAM") as dram,
        ):
            c.xin, c.etp, c.small = xin, etp, small
            c.psA, c.psB = psA, psB

            # ---- constants (k weights first: they gate the critical path;
            # the 2MB wp load is deferred into the loop so it doesn't hold
            # the shared DMA engines ahead of chunk 0) ----
            c.wk_sb = const.tile([128, CT, 128], BF16, name="wk_sb")
            c.wq_sb = const.tile([128, CT, 128], BF16, name="wq_sb")
            c.wv_sb = const.tile([128, CT, 128], BF16, name="wv_sb")
            c.wp_sb = const.tile([128, CT, DIM], BF16, name="wp_sb")
            c.bmat_sb = const.tile([128, CT], F32, name="bmat_sb")
            c.ones_sb = const.tile([DH + 1, DH], BF16, name="ones_sb")
            nc.sync.dma_start(c.wk_sb[:],
                              wkT_d.ap().rearrange("p (a m) -> p a m", a=CT))
            nc.vector.memset(c.ones_sb[DH:DH + 1, :], 1.0)

            # persistent activations
            c.qT = acts.tile([128, T], BF16, name="qT")
            c.kT = acts.tile([128, T], BF16, name="kT")
            c.v_aug = [acts.tile([128, T // 128, DH + 1], BF16,
                                 name=f"v_aug{h}") for h in range(HPC)]
            c.aT = [acts.tile([DH, T], BF16, name=f"aT{h}")
                    for h in range(HPC)]
            c.agT = acts.tile([128, CT, TPC], BF16, name="agT")
            c.y_sb = acts.tile([128, CT, TPC], BF16, name="y_sb")

            for h in range(HPC):
                nc.vector.memset(c.v_aug[h][:, :, DH:DH + 1], 1.0)

            # warmup: a few dummy matmuls raise the PE HAM clock gate to
            # 8/8 and a dummy exp preloads the ACT table set, all during
            # the initial x DMA wait.
            warm = acts.tile([128, 512], BF16, name="warm")
            c.warm = warm
            nc.vector.memset(warm[:], 0.0)
            wm_ps = psA.tile([128, 512], F32, tag="psA", name="wm_ps")
            for _w in range(14):
                nc.tensor.matmul(wm_ps[:], warm[:, 0:128], warm[:],
                                 start=(_w == 0), stop=(_w == 13))
            we_t = etp.tile([128, 512], BF16, tag="et", name="we_t")
            nc.scalar.activation(we_t[:], wm_ps[:],
                                 mybir.ActivationFunctionType.Exp)

            c.a2a_in = [[dram.tile([N_CORES, HPC, DH, SPP], BF16,
                                   name=f"a2a_in{b}{hf}") for hf in range(2)]
                        for b in range(B)]
            c.a2a_out = [[dram.tile([N_CORES, HPC, DH, SPP], BF16,
                                    name=f"a2a_out{b}{hf}") for hf in range(2)]
                         for b in range(B)]

            c.pending_norm = None
            c.prefetch = {}
            for _rep in range(reps):
                # batch 0: chunk 0's k/q/v, then attention nc0 with the
                # remaining b0 chunks interleaved at m-tile granularity
                # (QK^T of m-tile mt needs k of chunk mt//4).
                xcs = {0: _load_chunk(c, 0)}
                if _rep == 0:
                    # remaining small weights ride behind chunk 0 on the
                    # DMA engines: only wk gates the first matmul. The 2MB
                    # wp load is deferred until every x chunk's DMA has
                    # been issued (it once sat between chunk 0 and chunk 1
                    # on the SP ring and stalled batch-0 for its full 6us
                    # transfer); it is only needed ~100us later.
                    nc.sync.dma_start(
                        c.wq_sb[:],
                        wqT_d.ap().rearrange("p (a m) -> p a m", a=CT))
                    nc.sync.dma_start(
                        c.wv_sb[:],
                        wvT_d.ap().rearrange("p (a m) -> p a m", a=CT))
                    nc.sync.dma_start(c.bmat_sb[:], bmat_d[:])

                def _load_wp(rep):
                    def f():
                        if rep == 0:
                            nc.sync.dma_start(
                                c.wp_sb[:],
                                wpT_d.ap().rearrange("p (a m) -> p a m", a=CT))
                    return f
                _k_chunk(c, xcs[0], 0)
                _qv_chunk(c, xcs[0], 0)

                def _mk(tci, drop):
                    def f():
                        xcs[tci] = _load_chunk(c, tci)
                        _k_chunk(c, xcs[tci], tci)
                        _qv_chunk(c, xcs[tci], tci)
                        if drop in xcs:
                            xcs.pop(drop)
                    return f

                def _mk_load(tci):
                    def f():
                        xcs[tci] = _load_chunk(c, tci)
                    return f

                def _mk_compute(tci, drop):
                    def f():
                        _k_chunk(c, xcs[tci], tci)
                        _qv_chunk(c, xcs[tci], tci)
                        if drop in xcs:
                            xcs.pop(drop)
                    return f
                _attn_nchunk(c, 0, 0,
                             interleave={0: _mk(1, -1), 4: _mk(2, 0),
                                         8: _mk(3, 1)})
                # batch-1 loads prefetch well ahead of their compute so the
                # last k-chunk never gates batch-1's first QK^T
                _attn_nchunk(c, 0, 1, interleave={
                    1: _mk(4, 2), 4: _mk_load(5), 7: _mk_load(6),
                    9: _mk_compute(5, 3), 12: _mk_load(7),
                    14: _load_wp(_rep)})
                _attn_nchunk(c, 0, 2, interleave={1: _mk_compute(6, 4)})
                _attn_nchunk(c, 0, 3, interleave={1: _mk_compute(7, 5)})

                # batch-1 attention runs pure (Act-paced, no interleaved
                # projection bursts to stretch its exp pipeline)
                _attn_nchunk(c, 1, 0)
                _attn_nchunk(c, 1, 1)
                _attn_nchunk(c, 1, 2)
                _attn_nchunk(c, 1, 3)
                # flush b1-nci3's normalize (this also stages + fires the
                # final AllToAll), then run the ENTIRE projection on PE
                # inside the final AllToAll's latency window: every agT
                # gather except b1-half1's completed chunks ago.
                _norm_apply(c, c.pending_norm)
                c.pending_norm = None
                _proj_cols(c, 0, SPP, range(CT))
                _proj_cols(c, SPP, SPP, range(CT))
                _out_write(c, 0, SPB)
                _proj_cols(c, SPB, SPP, range(CT))
                _out_write(c, SPB, SPP)
                _proj_cols(c, SPB + SPP, SPP, range(0, 4))
                _out_write(c, SPB + SPP, SPP, ots=(0, 4))
                _proj_cols(c, SPB + SPP, SPP, range(4, CT))
                _out_write(c, SPB + SPP, SPP, ots=(4, CT))

    nc.compile()
    return nc


def _pmajor(w):
    """[DIM, M] (c-dim major) -> [128, CT*M] partition-major layout."""
    m = w.shape[1]
    return np.ascontiguousarray(
        w.reshape(CT, 128, m).transpose(1, 0, 2).reshape(128, CT * m))


def _prep_inputs(x, w_qkv, w_proj, b_proj):
    xf = np.ascontiguousarray(x.reshape(T, DIM).T).astype(ml_dtypes.bfloat16)
    wpT = _pmajor(np.ascontiguousarray(w_proj.T)).astype(ml_dtypes.bfloat16)
    bmat = np.ascontiguousarray(b_proj.reshape(CT, 128).T).astype(np.float32)
    in_maps = []
    for c in range(N_CORES):
        r0 = 128 * c
        wqT = _pmajor(w_qkv[r0:r0 + 128, :].T).astype(ml_dtypes.bfloat16)
        wkT = _pmajor(
            w_qkv[DIM + r0:DIM + r0 + 128, :].T).astype(ml_dtypes.bfloat16)
        wvT = _pmajor(
            w_qkv[2 * DIM + r0:2 * DIM + r0 + 128, :].T).astype(ml_dtypes.bfloat16)
        in_maps.append({
            "xT": xf, "wqT": wqT, "wkT": wkT, "wvT": wvT,
            "wpT": wpT, "bmat": bmat,
        })
    return in_maps


def _assemble(results):
    out = np.empty((T, DIM), dtype=np.float32)
    for c in range(N_CORES):
        yT = np.asarray(results[c]["out"], dtype=np.float32)  # [DIM, TPC]
        for b in range(B):
            for hf in range(2):
                t0 = b * N + hf * (N // 2) + c * SPP
                col0 = b * SPB + hf * SPP
                out[t0:t0 + SPP, :] = yT[:, col0:col0 + SPP].T
    return out.reshape(B, N, DIM)


def kernel(x, w_qkv, w_proj, b_proj):
    global _cached
    x = np.asarray(x, dtype=np.float32)
    w_qkv = np.asarray(w_qkv, dtype=np.float32)
    w_proj = np.asarray(w_proj, dtype=np.float32)
    b_proj = np.asarray(b_proj, dtype=np.float32)

    if _cached is None:
        _cached = _build()
    nc = _cached

    in_maps = _prep_inputs(x, w_qkv, w_proj, b_proj)
    # the axon terminal occasionally reports a transient device wedge
    # (NRT_EXEC_UNIT_UNRECOVERABLE / mesh desynced) that clears on retry
    last = None
    for attempt in range(3):
        try:
            res = bass_utils.run_bass_kernel_spmd(
                nc, in_maps, core_ids=list(range(N_CORES)))
            return _assemble(res.results)
        except Exception as e:  # noqa: BLE001
            last = e
            import time as _time
            _time.sleep(5 * (attempt + 1))
    raise last


if __name__ == "__main__":
    import jax
    with jax.default_device(jax.devices("cpu")[0]):
        import reference
        inputs = {k: np.asarray(v) for k, v in reference.setup_inputs().items()}
        expected = np.asarray(reference.reference(**inputs))
    actual = kernel(**inputs)
    err = np.linalg.norm(actual - expected) / np.linalg.norm(expected)
    print("Relative error:", err)



# revision 3
# speedup vs baseline: 2.8822x; 2.8822x over previous
"""Distributed multi-head attention kernel for 8 TRN2 NeuronCores (v2).

Problem: x[2,2048,1024] -> qkv proj (w_qkv[3072,1024]) -> 16-head SDPA ->
out proj (w_proj[1024,1024], b_proj[1024]).

Sharding: tensor-parallel over heads. Core c owns heads {2c, 2c+1}:
  - stage 1 (per core): q/k/v for its 2 heads over ALL 4096 tokens,
    transposed score tiles S^T[m,n] per (batch, head), exp on the scalar
    engine (no max-subtraction: scores ~ N(0,1), fp32 exp is safe), PV with
    a trailing ones-column in V so PSUM row DH accumulates the softmax
    denominator, then normalize. Result: aT_h [64 head-dims, 4096 tokens].
  - Four AllToAlls (one per half-batch, all but the last overlapped with
    compute) reshard from head-parallel to token-parallel.
  - stage 2 (per core): y^T[1024, 512] = w_proj @ a + b_proj for its shard.

v2 changes vs v1 (all bf16 on the TensorE; fp8 was tried and rejected —
softmax weight noise passes ~1:1 to the output, it does NOT average out):
  * The Activation ring issues NO DMAs: exp is the only Act-engine work.
    Previously a collective-gated agT read on the Act ring head-of-line
    blocked the exps of the next batch, stalling the whole pipeline for
    the full AllToAll duration (~20 us, twice).
  * All DMAs ride the SP ring (x, weights partition-major with the 2MB
    wp load deferred past all x-chunk issues, consolidated a2a-in
    writes, split agT gathers, consolidated y writes); the Pool ring
    carries ONLY the AllToAlls.
  * gpsimd partition_broadcast is replaced by a 1-contraction PE matmul
    (ones stationary at tile_position (64,0)) so the normalize never
    queues behind an in-flight collective on the Pool ring; it is
    deferred into m-tile 2 of the next chunk so nothing waits on the
    DVE reciprocal.
  * The ENTIRE output projection runs at the tail, inside the final
    AllToAll's latency window (the PE is otherwise idle there and every
    agT gather except the final one completed chunks earlier).
    Interleaving proj bursts into batch-1's attention instead stretches
    its Act-paced exp pipeline chunk for chunk, and proj pieces that
    outrun their agT data wedge the PE's 4-deep wait queue.

All TensorE matmuls run in bf16 (fp32 PSUM accumulation); softmax exp is
computed in fp32 on the scalar engine straight out of PSUM; 1/denominator
is bf16 (0.2% scale noise, inside the error budget).
"""

import numpy as np
import ml_dtypes

import concourse.bass as bass
import concourse.bacc as bacc
import concourse.tile as tile
import concourse.mybir as mybir
from concourse import bass_utils

BF16 = mybir.dt.bfloat16
F32 = mybir.dt.float32
FP8 = mybir.dt.float8e4
DR = mybir.MatmulPerfMode.DoubleRow
NP_FP8 = ml_dtypes.float8_e4m3

N_CORES = 8
B = 2
N = 2048
DIM = 1024
H = 16
DH = 64
SCALE = DH ** -0.5
HPC = H // N_CORES          # heads per core = 2
T = B * N                   # 4096 global tokens
TPC = T // N_CORES          # 512 tokens per core in stage 2
SPB = TPC // B              # stage-2 tokens per core per batch = 256
SPP = SPB // 2              # tokens per shard piece = 128
CT = DIM // 128             # 8 contraction tiles
CP = CT // 2                # 4 contraction pair-tiles for DoubleRow
TCH = 512                   # token chunk for stage-1 matmul streaming
NCH = 512                   # n (query) chunk in attention
MT = N // 128               # 16 m-tiles per batch

_cached = None


class _Ctx:
    pass


def _load_chunk(c, tci):
    """DMA one token chunk of x^T into SBUF, split per c-tile pair so the
    first matmul can start after 1/4 of the transfer."""
    t0 = tci * TCH
    xc = c.xin.tile([128, CT, TCH], BF16, tag="xc", name="xc")
    for a in range(CP):
        c.nc.sync.dma_start(
            xc[:, 2 * a:2 * a + 2, :],
            c.xT_d[256 * a:256 * (a + 1), t0:t0 + TCH]
            .rearrange("(t p) n -> p t n", p=128))
    return xc


def _k_chunk(c, xc, tci):
    t0 = tci * TCH
    k_ps = c.psA.tile([128, TCH], F32, tag="psA", name="k_ps")
    for a in range(CT):
        c.nc.tensor.matmul(k_ps[:], c.wk_sb[:, a, :], xc[:, a, :],
                           start=(a == 0), stop=(a == CT - 1))
    c.nc.vector.tensor_copy(c.kT[:, t0:t0 + TCH], k_ps[:])


def _q_chunk(c, xc, tci):
    t0 = tci * TCH
    q_ps = c.psA.tile([128, TCH], F32, tag="psA", name="q_ps")
    for a in range(CT):
        c.nc.tensor.matmul(q_ps[:], c.wq_sb[:, a, :], xc[:, a, :],
                           start=(a == 0), stop=(a == CT - 1))
    c.nc.vector.tensor_copy(c.qT[:, t0:t0 + TCH], q_ps[:])


def _v_chunk(c, xc, tci):
    t0 = tci * TCH
    for mt in range(TCH // 128):
        gmt = (t0 // 128) + mt
        v_ps = c.psA.tile([128, 128], F32, tag="psA", name="v_ps")
        for a in range(CT):
            c.nc.tensor.matmul(
                v_ps[:], xc[:, a, 128 * mt:128 * (mt + 1)], c.wv_sb[:, a, :],
                start=(a == 0), stop=(a == CT - 1))
        for h in range(HPC):
            c.nc.vector.tensor_copy(
                c.v_aug[h][:, gmt, 0:DH], v_ps[:, DH * h:DH * (h + 1)])


def _qv_chunk(c, xc, tci):
    _q_chunk(c, xc, tci)
    _v_chunk(c, xc, tci)


def _attn_qk_exp(c, b, nci, mt):
    nc = c.nc
    n0 = b * N + nci * NCH
    m0 = b * N + 128 * mt
    s_ps = c.psA.tile([128, HPC * NCH], F32, tag="psA", name="s_ps")
    e_t = c.etp.tile([128, HPC * NCH], BF16, tag="et", name="e_t")
    for h in range(HPC):
        nc.tensor.matmul(
            s_ps[:, NCH * h:NCH * (h + 1)],
            c.kT[DH * h:DH * (h + 1), m0:m0 + 128],
            c.qT[DH * h:DH * (h + 1), n0:n0 + NCH],
            start=True, stop=True,
            tile_position=(DH * h, 0))
    nc.scalar.activation(e_t[:], s_ps[:],
                         mybir.ActivationFunctionType.Exp, scale=SCALE)
    return e_t


def _attn_pv(c, mt, gmt, e_t, o_ps):
    nc = c.nc
    for h in range(HPC):
        nc.tensor.matmul(
            o_ps[h][:], c.v_aug[h][:, gmt, :],
            e_t[:, NCH * h:NCH * (h + 1)],
            start=(mt == 0), stop=(mt == MT - 1))


def _norm_recip(c, b, nci, o_ps):
    """Row DH of o_ps is the softmax denominator (ones-column of v_aug);
    the reciprocal stays at partition 64 — engine lanes are physical, so
    in/out partition ranges must match and start 0/32/64/96-aligned.
    bf16 out: a 0.2% scale noise per token, inside the error budget, and
    it keeps the broadcast matmul at full bf16 speed."""
    nc = c.nc
    den = c.small.tile([DH + 1, HPC * NCH], BF16, tag="den", name="den")
    with nc.allow_low_precision(reason="bf16 1/den: 0.2% scale noise ok"):
        for h in range(HPC):
            nc.vector.reciprocal(den[DH:DH + 1, NCH * h:NCH * (h + 1)],
                                 o_ps[h][DH:DH + 1, :])
    return den


def _norm_apply(c, pend):
    """Broadcast 1/den from partition 64 across partitions 0..DH-1 with a
    1-contraction PE matmul at tile_position (64, 0) (no gpsimd: the Pool
    ring is collectives-only so AllToAlls never head-of-line block the
    broadcast; no DMA: the Act ring stays exp-only), then scale rows
    0..DH-1 of o_ps into aT. Deferred into m-tile 2 of the next chunk so
    the PE matmul never waits on the DVE reciprocal."""
    nc = c.nc
    b, nci, o_ps, den = pend
    n0 = b * N + nci * NCH
    rb = c.small.tile([DH, HPC * NCH], F32, tag="rb", name="rb")
    rb_ps = c.psA.tile([DH, HPC, NCH], F32, tag="psA", name="rb_ps")
    for h in range(HPC):
        nc.tensor.matmul(rb_ps[:, h, :], c.ones_sb[DH:DH + 1, :],
                         den[DH:DH + 1, NCH * h:NCH * (h + 1)],
                         start=True, stop=True, tile_position=(DH, 0))
    # DVE reads at most one PSUM operand per op: stage rb in SBUF,
    # per head so each mul (and the a2a write behind it) starts as soon
    # as its half of the staging copy lands
    for h in range(HPC):
        nc.vector.tensor_copy(rb[:, NCH * h:NCH * (h + 1)], rb_ps[:, h, :])
        nc.vector.tensor_mul(c.aT[h][:, n0:n0 + NCH],
                             o_ps[h][0:DH, :],
                             rb[:, NCH * h:NCH * (h + 1)])
    # aT columns for this chunk are final: stage its reshard pieces
    half, sub = divmod(nci, 2)
    if sub == 1:
        _a2a_in_writes(c, b, half)
        _reshard_half(c, b, half)


def _prefetch_qk(c, b, nci, mt):
    """Compute one of a later n-chunk's QK^T+exp tiles now, in this
    chunk's Activation-idle window, and stash it for the PV loop."""
    def f():
        c.prefetch[(b, nci, mt)] = _attn_qk_exp(c, b, nci, mt)
    return f


def _attn_nchunk(c, b, nci, interleave=None):
    """One attention n-chunk; optional {mt: fn} callbacks emitted after
    given m-tiles to fill TensorEngine slack. The previous chunk's
    normalize (PE broadcast + DVE scale) is emitted after m-tile 1."""
    o_ps = [c.psB.tile([DH + 1, NCH], F32, tag="psB", name=f"o_ps{h}")
            for h in range(HPC)]
    # software-pipelined two deep: PV of m-tile mt-2 is emitted after
    # QK^T/exp of m-tile mt, so the next QK^T never queues behind an
    # exp-blocked PV, and the first PV (which waits for the previous
    # chunk's o_ps slot, released by its normalize-muls) has two QK^T
    # pairs in front of it to cover the normalize latency.
    pv_q = []
    for mt in range(MT):
        e_t = c.prefetch.pop((b, nci, mt), None)
        if e_t is None:
            e_t = _attn_qk_exp(c, b, nci, mt)
        if mt == 2 and c.pending_norm is not None:
            _norm_apply(c, c.pending_norm)
            c.pending_norm = None
        if len(pv_q) >= 4:
            pmt, pe_t = pv_q.pop(0)
            _attn_pv(c, pmt, b * MT + pmt, pe_t, o_ps)
        pv_q.append((mt, e_t))
        if interleave and mt in interleave:
            interleave[mt]()
    for pmt, pe_t in pv_q:
        _attn_pv(c, pmt, b * MT + pmt, pe_t, o_ps)
    den = _norm_recip(c, b, nci, o_ps)
    c.pending_norm = (b, nci, o_ps, den)


def _a2a_in_writes(c, b, half):
    """Write this half-batch's aT columns to the a2a staging buffer: one
    DMA per head on the DVE ring (dest iterates (d, j, t) to match the
    natural SBUF source order)."""
    t0 = b * N + half * (N // 2)
    for h in range(HPC):
        c.nc.sync.dma_start(
            c.a2a_in[b][half][:, h, :, :].rearrange("j d t -> d j t"),
            c.aT[h][:, t0:t0 + N // 2])


def _reshard_half(c, b, half):
    """AllToAll for half-batch (b, half): core j gets C-complete rows for
    its SPP-token piece [b*N + half*N/2 + SPP*j, +SPP). The agT gather
    rides the SP ring so it never blocks exps behind the collective."""
    nc = c.nc
    if c.use_collective:
        nc.gpsimd.collective_compute(
            "AllToAll", mybir.AluOpType.bypass,
            replica_groups=[list(range(N_CORES))],
            ins=[c.a2a_in[b][half].opt()], outs=[c.a2a_out[b][half].opt()])
    else:
        nc.sync.dma_start(c.a2a_out[b][half][:], c.a2a_in[b][half][:])
    col0 = b * SPB + half * SPP
    # split the gather so the projection's first a-tiles can start after
    # a fraction of the transfer; the final (fully latency-exposed)
    # gather is split 4 ways
    ns = 4 if (b, half) == (1, 1) else 2
    step = CT // ns
    for i in range(ns):
        nc.sync.dma_start(
            c.agT[:, i * step:(i + 1) * step, col0:col0 + SPP],
            c.a2a_out[b][half][i * step:(i + 1) * step]
            .rearrange("w h d t -> (h d) w t"))


def _proj_cols(c, col0, ncols, ots):
    """Output projection for agT columns [col0, col0+ncols) into y_sb."""
    nc = c.nc
    for ot in ots:
        y_ps = c.psA.tile([128, SPB], F32, tag="psA", name="y_ps")
        for a in range(CT):
            nc.tensor.matmul(
                y_ps[:, 0:ncols], c.wp_sb[:, a, 128 * ot:128 * (ot + 1)],
                c.agT[:, a, col0:col0 + ncols],
                start=(a == 0), stop=(a == CT - 1))
        nc.vector.tensor_scalar_add(c.y_sb[:, ot, col0:col0 + ncols],
                                    y_ps[:, 0:ncols],
                                    c.bmat_sb[:, ot:ot + 1])


def _out_write(c, col0, ncols, ots=(0, CT)):
    o0, o1 = ots
    c.nc.sync.dma_start(
        c.out_d[128 * o0:128 * o1, col0:col0 + ncols]
        .rearrange("(o p) t -> p o t", p=128),
        c.y_sb[:, o0:o1, col0:col0 + ncols])


def _build(use_collective=True, reps=1, num_devices=None, attn_only=False):
    """reps>1 unrolls the whole computation N times inside one NEFF —
    used only for timing (differencing out per-execution overhead).
    attn_only=True skips reshard+proj (timing ablation only)."""
    if num_devices is None:
        num_devices = N_CORES if use_collective else 1
    nc = bacc.Bacc("TRN2", target_bir_lowering=False, debug=False,
                   num_devices=num_devices)
    c = _Ctx()
    c.nc = nc
    c.use_collective = use_collective

    c.xT_d = nc.dram_tensor("xT", [DIM, T], BF16, kind="ExternalInput")
    # weights arrive partition-major ([128, CT*...]) so their DMAs are 128
    # full-row descriptors instead of 1024 sub-512B ones
    wqT_d = nc.dram_tensor("wqT", [128, CT * 128], BF16, kind="ExternalInput")
    wkT_d = nc.dram_tensor("wkT", [128, CT * 128], BF16, kind="ExternalInput")
    wvT_d = nc.dram_tensor("wvT", [128, CT * 128], BF16, kind="ExternalInput")
    wpT_d = nc.dram_tensor("wpT", [128, CT * DIM], BF16, kind="ExternalInput")
    bmat_d = nc.dram_tensor("bmat", [128, CT], F32, kind="ExternalInput")
    c.out_d = nc.dram_tensor("out", [DIM, TPC], BF16, kind="ExternalOutput")

    with tile.TileContext(nc) as tc:
        with (
            tc.tile_pool(name="const", bufs=1) as const,
            tc.tile_pool(name="xin", bufs=7) as xin,
            tc.tile_pool(name="acts", bufs=1) as acts,
            tc.tile_pool(name="et", bufs=16) as etp,
            tc.tile_pool(name="small", bufs=3) as small,
            tc.tile_pool(name="psA", bufs=3, space="PSUM") as psA,
            tc.tile_pool(name="psB", bufs=2, space="PSUM") as psB,
            tc.tile_pool(name="dram", bufs=1, space="DRAM") as dram,
        ):
            c.xin, c.etp, c.small = xin, etp, small
            c.psA, c.psB = psA, psB

            # ---- constants (k weights first: they gate the critical path;
            # the 2MB wp load is deferred into the loop so it doesn't hold
            # the shared DMA engines ahead of chunk 0) ----
            c.wk_sb = const.tile([128, CT, 128], BF16, name="wk_sb")
            c.wq_sb = const.tile([128, CT, 128], BF16, name="wq_sb")
            c.wv_sb = const.tile([128, CT, 128], BF16, name="wv_sb")
            c.wp_sb = const.tile([128, CT, DIM], BF16, name="wp_sb")
            c.bmat_sb = const.tile([128, CT], F32, name="bmat_sb")
            c.ones_sb = const.tile([DH + 1, DH], BF16, name="ones_sb")
            nc.sync.dma_start(c.wk_sb[:],
                              wkT_d.ap().rearrange("p (a m) -> p a m", a=CT))
            nc.vector.memset(c.ones_sb[DH:DH + 1, :], 1.0)

            # persistent activations
            c.qT = acts.tile([128, T], BF16, name="qT")
            c.kT = acts.tile([128, T], BF16, name="kT")
            c.v_aug = [acts.tile([128, T // 128, DH + 1], BF16,
                                 name=f"v_aug{h}") for h in range(HPC)]
            c.aT = [acts.tile([DH, T], BF16, name=f"aT{h}")
                    for h in range(HPC)]
            c.agT = acts.tile([128, CT, TPC], BF16, name="agT")
            c.y_sb = acts.tile([128, CT, TPC], BF16, name="y_sb")

            for h in range(HPC):
                nc.vector.memset(c.v_aug[h][:, :, DH:DH + 1], 1.0)

            # warmup: a few dummy matmuls raise the PE HAM clock gate to
            # 8/8 and a dummy exp preloads the ACT table set, all during
            # the initial x DMA wait.
            warm = acts.tile([128, 512], BF16, name="warm")
            c.warm = warm
            nc.vector.memset(warm[:], 0.0)
            wm_ps = psA.tile([128, 512], F32, tag="psA", name="wm_ps")
            for _w in range(14):
                nc.tensor.matmul(wm_ps[:], warm[:, 0:128], warm[:],
                                 start=(_w == 0), stop=(_w == 13))
            we_t = etp.tile([128, 512], BF16, tag="et", name="we_t")
            nc.scalar.activation(we_t[:], wm_ps[:],
                                 mybir.ActivationFunctionType.Exp)

            c.a2a_in = [[dram.tile([N_CORES, HPC, DH, SPP], BF16,
                                   name=f"a2a_in{b}{hf}") for hf in range(2)]
                        for b in range(B)]
            c.a2a_out = [[dram.tile([N_CORES, HPC, DH, SPP], BF16,
                                    name=f"a2a_out{b}{hf}") for hf in range(2)]
                         for b in range(B)]

            c.pending_norm = None
            c.prefetch = {}
            for _rep in range(reps):
                # batch 0: chunk 0's k/q/v, then attention nc0 with the
                # remaining b0 chunks interleaved at m-tile granularity
                # (QK^T of m-tile mt needs k of chunk mt//4).
                xcs = {0: _load_chunk(c, 0)}
                if _rep == 0:
                    # remaining small weights ride behind chunk 0 on the
                    # DMA engines: only wk gates the first matmul. The 2MB
                    # wp load is deferred until every x chunk's DMA has
                    # been issued (it once sat between chunk 0 and chunk 1
                    # on the SP ring and stalled batch-0 for its full 6us
                    # transfer); it is only needed ~100us later.
                    nc.sync.dma_start(
                        c.wq_sb[:],
                        wqT_d.ap().rearrange("p (a m) -> p a m", a=CT))
                    nc.sync.dma_start(
                        c.wv_sb[:],
                        wvT_d.ap().rearrange("p (a m) -> p a m", a=CT))
                    nc.sync.dma_start(c.bmat_sb[:], bmat_d[:])

                def _load_wp(rep):
                    def f():
                        if rep == 0:
                            nc.sync.dma_start(
                                c.wp_sb[:],
                                wpT_d.ap().rearrange("p (a m) -> p a m", a=CT))
                    return f
                _k_chunk(c, xcs[0], 0)
                _qv_chunk(c, xcs[0], 0)

                def _mk(tci, drop):
                    def f():
                        xcs[tci] = _load_chunk(c, tci)
                        _k_chunk(c, xcs[tci], tci)
                        _qv_chunk(c, xcs[tci], tci)
                        if drop in xcs:
                            xcs.pop(drop)
                    return f

                def _mk_load(tci):
                    def f():
                        xcs[tci] = _load_chunk(c, tci)
                    return f

                def _mk_compute(tci, drop):
                    def f():
                        _k_chunk(c, xcs[tci], tci)
                        _qv_chunk(c, xcs[tci], tci)
                        if drop in xcs:
                            xcs.pop(drop)
                    return f
                _attn_nchunk(c, 0, 0,
                             interleave={0: _mk(1, -1), 4: _mk(2, 0),
                                         8: _mk(3, 1)})
                # batch-1 loads prefetch well ahead of their compute so the
                # last k-chunk never gates batch-1's first QK^T
                _attn_nchunk(c, 0, 1, interleave={
                    1: _mk(4, 2), 4: _mk_load(5), 7: _mk_load(6),
                    9: _mk_compute(5, 3), 12: _mk_load(7),
                    14: _load_wp(_rep)})
                _attn_nchunk(c, 0, 2, interleave={1: _mk_compute(6, 4)})
                _attn_nchunk(c, 0, 3, interleave={1: _mk_compute(7, 5)})

                # batch-1 attention runs pure (Act-paced, no interleaved
                # projection bursts to stretch its exp pipeline)
                _attn_nchunk(c, 1, 0)
                _attn_nchunk(c, 1, 1)
                _attn_nchunk(c, 1, 2)
                _attn_nchunk(c, 1, 3)
                # flush b1-nci3's normalize (this also stages + fires the
                # final AllToAll), then run the ENTIRE projection on PE
                # inside the final AllToAll's latency window: every agT
                # gather except b1-half1's completed chunks ago.
                _norm_apply(c, c.pending_norm)
                c.pending_norm = None
                _proj_cols(c, 0, SPP, range(CT))
                _proj_cols(c, SPP, SPP, range(CT))
                _out_write(c, 0, SPB)
                _proj_cols(c, SPB, SPP, range(CT))
                _out_write(c, SPB, SPP)
                _proj_cols(c, SPB + SPP, SPP, range(0, 4))
                _out_write(c, SPB + SPP, SPP, ots=(0, 4))
                _proj_cols(c, SPB + SPP, SPP, range(4, CT))
                _out_write(c, SPB + SPP, SPP, ots=(4, CT))

    nc.compile()
    return nc


def _pmajor(w):
    """[DIM, M] (c-dim major) -> [128, CT*M] partition-major layout."""
    m = w.shape[1]
    return np.ascontiguousarray(
        w.reshape(CT, 128, m).transpose(1, 0, 2).reshape(128, CT * m))


def _prep_inputs(x, w_qkv, w_proj, b_proj):
    xf = np.ascontiguousarray(x.reshape(T, DIM).T).astype(ml_dtypes.bfloat16)
    wpT = _pmajor(np.ascontiguousarray(w_proj.T)).astype(ml_dtypes.bfloat16)
    bmat = np.ascontiguousarray(b_proj.reshape(CT, 128).T).astype(np.float32)
    in_maps = []
    for c in range(N_CORES):
        r0 = 128 * c
        wqT = _pmajor(w_qkv[r0:r0 + 128, :].T).astype(ml_dtypes.bfloat16)
        wkT = _pmajor(
            w_qkv[DIM + r0:DIM + r0 + 128, :].T).astype(ml_dtypes.bfloat16)
        wvT = _pmajor(
            w_qkv[2 * DIM + r0:2 * DIM + r0 + 128, :].T).astype(ml_dtypes.bfloat16)
        in_maps.append({
            "xT": xf, "wqT": wqT, "wkT": wkT, "wvT": wvT,
            "wpT": wpT, "bmat": bmat,
        })
    return in_maps


def _assemble(results):
    out = np.empty((T, DIM), dtype=np.float32)
    for c in range(N_CORES):
        yT = np.asarray(results[c]["out"], dtype=np.float32)  # [DIM, TPC]
        for b in range(B):
            for hf in range(2):
                t0 = b * N + hf * (N // 2) + c * SPP
                col0 = b * SPB + hf * SPP
                out[t0:t0 + SPP, :] = yT[:, col0:col0 + SPP].T
    return out.reshape(B, N, DIM)


def kernel(x, w_qkv, w_proj, b_proj):
    global _cached
    x = np.asarray(x, dtype=np.float32)
    w_qkv = np.asarray(w_qkv, dtype=np.float32)
    w_proj = np.asarray(w_proj, dtype=np.float32)
    b_proj = np.asarray(b_proj, dtype=np.float32)

    if _cached is None:
        _cached = _build()
    nc = _cached

    in_maps = _prep_inputs(x, w_qkv, w_proj, b_proj)
    # the axon terminal occasionally reports a transient device wedge
    # (NRT_EXEC_UNIT_UNRECOVERABLE / mesh desynced) that clears on retry
    last = None
    for attempt in range(3):
        try:
            res = bass_utils.run_bass_kernel_spmd(
                nc, in_maps, core_ids=list(range(N_CORES)))
            return _assemble(res.results)
        except Exception as e:  # noqa: BLE001
            last = e
            import time as _time
            _time.sleep(5 * (attempt + 1))
    raise last


if __name__ == "__main__":
    import jax
    with jax.default_device(jax.devices("cpu")[0]):
        import reference
        inputs = {k: np.asarray(v) for k, v in reference.setup_inputs().items()}
        expected = np.asarray(reference.reference(**inputs))
    actual = kernel(**inputs)
    err = np.linalg.norm(actual - expected) / np.linalg.norm(expected)
    print("Relative error:", err)

